# revision 17
# baseline (speedup 1.0000x reference)
"""Trainium2 Bass kernel for BLIF spiking-neuron layer.

Math: the reference's zero-padded-FFT causal conv with kernel
exp(-a_c * t) is exactly the first-order linear recurrence

    v[t] = lam_c * v[t-1] + x[t],   lam_c = exp(-exp(A_log_c))

followed by spike + refractory masking, which reduces to

    s[t]   = (v[t] > 1)
    out[t] = s[t] * (1 - s[t-1])  =  (s[t-1] < s[t])

Sharding: batch B=8 -> one batch per NeuronCore (8 cores), no
cross-core communication. Per core: x_b [T=256, C=128, F=196] f32.

Per-core layout: C on partitions, F(=H*W) on the free dim, T as the
sequential axis. The recurrence runs as 256 dependent
scalar_tensor_tensor ops on VectorE; spikes are encoded per chunk as
sign(v-1) in bf16 on ScalarE; the refractory mask is a shifted bf16
is_lt compare on VectorE (2x mode); spikes stream out as bf16 and are
converted to f32 on the host.
"""

import sys

for _p in ("/opt/trn_rl_repo", "/root/.axon_site/_ro/trn_rl_repo"):
    if _p not in sys.path:
        sys.path.append(_p)

import numpy as np

import concourse.bacc as bacc
import concourse.bass as bass
import concourse.mybir as mybir
import concourse.tile as tile
from concourse.bass_utils import run_bass_kernel_spmd

T, B, C, H, W = 256, 8, 128, 14, 14
F = H * W          # 196
TC = 32            # timesteps per chunk
NCH = T // TC      # 8 chunks
N_CORES = 8

f32 = mybir.dt.float32
bf16 = mybir.dt.bfloat16
Alu = mybir.AluOpType

_cached_nc = None


def build_program():
    global _cached_nc
    if _cached_nc is not None:
        return _cached_nc

    # Bacc (not raw Bass): its finalize() runs generate_event_semaphores,
    # which splits multi-wait instructions to satisfy the TRN2 limit of one
    # sync wait per instruction (walrus rejects the IR otherwise).
    nc = bacc.Bacc()
    x_ext = nc.declare_dram_parameter("x", [T, C, F], f32, isOutput=False)
    lam_ext = nc.declare_dram_parameter("lam", [C, 1], f32, isOutput=False)
    # Chunk-major output layout: each chunk's store footprint is one
    # contiguous DRAM range, so consecutive out-DMAs don't look WAW-
    # dependent to Tile's range-based tracker (which would exceed the
    # 1-sync-wait budget of the DMA instruction struct).
    out_ext = nc.declare_dram_parameter("out", [NCH, C, TC * F], bf16, isOutput=True)

    with tile.TileContext(nc) as tc:
        with (
            tc.tile_pool(name="singles", bufs=1) as singles,
            tc.tile_pool(name="xp", bufs=2) as xp,
            tc.tile_pool(name="vp", bufs=2) as vp,
            tc.tile_pool(name="sp", bufs=2) as sp,
            tc.tile_pool(name="op", bufs=2) as op,
            tc.tile_pool(name="xc", bufs=2) as xcp,
        ):
            # The S2S2D2_STT instruction struct only has room for ONE sync
            # wait, and Tile emits a same-engine DVE wait on every
            # RAW-dependent op — so no cross-engine (DMA/ACT) wait may land
            # on a scalar_tensor_tensor. Small DVE copies absorb the DMA
            # completion ticks into the DVE vector clock first.
            lam_dma = singles.tile([C, 1], f32)
            nc.sync.dma_start(lam_dma[:], lam_ext[:])
            lam_t = singles.tile([C, 1], f32)
            nc.vector.tensor_copy(out=lam_t[:], in_=lam_dma[:])
            neg1 = singles.tile([C, 1], f32)
            nc.vector.memset(neg1[:], -1.0)

            # boundary spike state for t = -1: v = 0 -> sign(v-1) = -1
            sinit = singles.tile([C, F], bf16)
            nc.vector.memset(sinit[:], -1.0)

            x_r = x_ext[:].rearrange("t c f -> c t f")

            prev_v = None
            prev_s = sinit[:]
            for k in range(NCH):
                x_t = xp.tile([C, TC, F], f32)
                nc.sync.dma_start(x_t[:], x_r[:, k * TC : (k + 1) * TC, :])

                # Absorb the x-DMA completion wait on a copy op so the first
                # recurrence STT below only needs its same-engine DVE wait.
                xc = xcp.tile([C, F], f32)
                nc.vector.tensor_copy(out=xc[:], in_=x_t[:, 0, :])

                v = vp.tile([C, TC * F], f32)
                src = prev_v
                for j in range(TC):
                    if src is None:
                        nc.vector.tensor_copy(out=v[:, 0:F], in_=xc[:])
                    else:
                        nc.vector.scalar_tensor_tensor(
                            out=v[:, j * F : (j + 1) * F],
                            in0=src,
                            scalar=lam_t[:],
                            in1=xc[:] if j == 0 else x_t[:, j, :],
                            op0=Alu.mult,
                            op1=Alu.add,
                        )
                    src = v[:, j * F : (j + 1) * F]

                # s = sign(v - 1) in {-1, 0, 1}, bf16-exact
                s = sp.tile([C, TC * F], bf16)
                nc.scalar.sign(s[:], v[:], bias=neg1[:])

                # out[t] = (s[t-1] < s[t])
                o = op.tile([C, TC * F], bf16)
                nc.vector.tensor_tensor(
                    out=o[:, 0:F], in0=prev_s, in1=s[:, 0:F], op=Alu.is_lt
                )
                nc.vector.tensor_tensor(
                    out=o[:, F : TC * F],
                    in0=s[:, 0 : (TC - 1) * F],
                    in1=s[:, F : TC * F],
                    op=Alu.is_lt,
                )

                nc.sync.dma_start(out_ext[k], o[:])

                prev_v = v[:, (TC - 1) * F : TC * F]
                prev_s = s[:, (TC - 1) * F : TC * F]

    nc.finalize()
    _cached_nc = nc
    return nc


def make_in_maps(x, A_log):
    lam = np.exp(-np.exp(A_log.astype(np.float64))).astype(np.float32).reshape(C, 1)
    return [
        {
            "x": np.ascontiguousarray(x[:, b]).reshape(T, C, F),
            "lam": lam,
        }
        for b in range(B)
    ]


def gather_output(results):
    outs = []
    for b in range(B):
        o = np.asarray(results[b]["out"])  # [NCH, C, TC*F] bf16
        o = o.astype(np.float32).reshape(NCH, C, TC, F)
        outs.append(o.transpose(0, 2, 1, 3).reshape(T, C, F))
    return np.stack(outs, axis=1).reshape(T, B, C, H, W)


def kernel(x, A_log):
    nc = build_program()
    in_maps = make_in_maps(x, A_log)
    res = run_bass_kernel_spmd(nc, in_maps, list(range(N_CORES)))
    return gather_output(res.results)
